# revision 37
# baseline (speedup 1.0000x reference)
"""Trainium2 Bass kernel for DensityGCNProcessor.

Model: 2-layer GCN over a per-sample kNN graph built from 1-D density values
(K=4 nearest by |density_i - density_j|), symmetric deg^-1/2 normalization on
target indegree, relu after each layer.

Strategy
--------
kNN in a 1-D metric means: after sorting nodes by density, every node's 4
nearest neighbours lie within +/-4 sorted positions, so aggregation is a
9-diagonal banded matrix in sorted order. The host does all index math
(argsort, band weights with exact reference tie-breaking) and also lays the
features out in sorted order, pre-tiled for the device: overlapping window
tiles of 128 sorted nodes at stride 120, so each band aggregation is a single
k=128 matmul (no halo matmul).

Device pipeline per core (all matmuls fp16, psum fp32):
  1. agg1  (chan-major): A1^T[cin,:] tiles = xs_tile^T @ bandT1_tile
  2. dense1: H^T = relu(W1^T A1^T + b1)   (chid-major, scalar/vector drains)
  3. dense2: T2 window tiles = (hT cols)^T @ W2   (node-major)
  4. agg2  (chan-major): out^T = relu(T2_tile^T @ bandT2_tile + b2)
  5. linear DMA of out^T [256, 2048]; host scatters columns back to the
     original node order while unsharding.

Sharding: 8 cores = 4 batches x 2 rank-halves. Core c handles batch c//2,
sorted ranks [ (c%2)*2048, (c%2)*2048+2048 ).
"""

import numpy as np

# ---------------------------------------------------------------- constants
B = 4
CIN = 256
CHID = 512
COUT = 256
H = W = 64
N = H * W            # 4096 nodes per batch
KNN = 4
BAND = 4             # kNN lies within +/-4 sorted positions
HALF = N // 2        # 2048 ranks per core
NT = 18              # window tiles (128 rows, stride 120)
TS = 120             # out columns per tile
NCOLS = NT * TS      # 2160 hT columns computed
NH = 2176            # hT allocated columns (tail zeroed)

_COMPILED = {}


# ---------------------------------------------------------------- host graph
def _build_band_weights(d_flat):
    """order [N], w9 [N, 9] f32: out_s[r] = sum_o w9[r, o+4] * g_s[r+o]."""
    order = np.argsort(d_flat, kind="stable")
    d_s = d_flat[order]

    offs = np.arange(-BAND, BAND + 1)
    ridx = np.arange(N)[:, None] + offs[None, :]
    valid = (ridx >= 0) & (ridx < N)
    ridx_c = np.clip(ridx, 0, N - 1)
    c = np.abs(d_s[ridx_c] - d_s[:, None]).astype(np.float32)
    c = np.where(valid, c, np.float32(np.inf))
    cand_j = np.where(valid, order[ridx_c], N)

    # reference = stable argsort over the full row: ties by smaller orig index.
    sel = np.lexsort((cand_j, c), axis=1)
    tgt_s = np.take_along_axis(ridx_c, sel[:, 1:KNN + 1], axis=1).reshape(-1)
    src_s = np.repeat(np.arange(N), KNN)

    deg = np.ones(N, dtype=np.float32)
    np.add.at(deg, tgt_s, np.float32(1.0))
    dinv = (np.float32(1.0) / np.sqrt(deg)).astype(np.float32)

    m = np.zeros((N, 9), dtype=np.float32)
    np.add.at(m, (tgt_s, src_s - tgt_s + BAND), np.float32(1.0))
    m[:, BAND] += 1.0  # self loops

    ro = np.arange(N)[:, None] + offs[None, :]
    rov = (ro >= 0) & (ro < N)
    w9 = m * dinv[:, None] * dinv[np.clip(ro, 0, N - 1)] * rov
    return order.astype(np.int64), w9.astype(np.float32)


def _host_graph(density_maps):
    """Per-core tensors. Returns list of 8 dicts + per-batch orders."""
    pidx = np.arange(128)[:, None, None]          # window row
    tidx = np.arange(NT)[None, :, None]           # tile
    ridx = np.arange(TS)[None, None, :]           # out col within tile
    oo = pidx - ridx                              # w9 column (offset + 4)
    ok_o = (oo >= 0) & (oo <= 8)
    oo_c = np.clip(oo, 0, 8)

    per_core, orders = [], []
    for b in range(B):
        d = np.asarray(density_maps[b]).reshape(N).astype(np.float32)
        order, w9 = _build_band_weights(d)
        orders.append(order)
        for half in range(2):
            r0 = half * HALF

            # layer-1 band tiles: out rank = r0 - 4 + 120 t + r
            rank1 = r0 - 4 + TS * tidx + ridx
            ok1 = ok_o & (rank1 >= 0) & (rank1 < N)
            bt1 = np.where(ok1, w9[np.clip(rank1, 0, N - 1), oo_c], 0.0)

            # layer-2 band tiles: out rank = r0 + 120 t + r, only first 2048
            rank2 = r0 + TS * tidx + ridx
            ok2 = ok_o & (TS * tidx + ridx < HALF) & (rank2 < N)
            bt2 = np.where(ok2, w9[np.clip(rank2, 0, N - 1), oo_c], 0.0)

            # sorted feature window tiles: row p of tile t = rank r0-8+120t+p
            gi = r0 - 8 + TS * np.arange(NT)[None, :] + np.arange(128)[:, None]
            node = order[np.clip(gi, 0, N - 1)]   # [128, NT]

            per_core.append(dict(
                bt1=bt1.astype(np.float16),
                bt2=bt2.astype(np.float16),
                node=node,
            ))
    return per_core, orders


# ---------------------------------------------------------------- device IR
def build_nc():
    import concourse.bacc as bacc
    import concourse.mybir as mybir
    from concourse.tile import TileContext

    F32 = mybir.dt.float32
    F16 = mybir.dt.float16
    Relu = mybir.ActivationFunctionType.Relu
    Copy = mybir.ActivationFunctionType.Copy
    add = mybir.AluOpType.add
    amax = mybir.AluOpType.max

    nc = bacc.Bacc()
    xs_d = nc.dram_tensor("xs", [128, NT, CIN], F16, kind="ExternalInput")
    bt1_d = nc.dram_tensor("bt1", [128, NT, TS], F16, kind="ExternalInput")
    bt2_d = nc.dram_tensor("bt2", [128, NT, TS], F16, kind="ExternalInput")
    w1_d = nc.dram_tensor("w1b", [128, 2, CHID], F16, kind="ExternalInput")
    w2_d = nc.dram_tensor("w2b", [128, 4, COUT], F16, kind="ExternalInput")
    b1_d = nc.dram_tensor("b1v", [128, 4], F32, kind="ExternalInput")
    b2_d = nc.dram_tensor("b2v", [128, 2], F32, kind="ExternalInput")
    outT_d = nc.dram_tensor("outT", [COUT, HALF], F16, kind="ExternalOutput")

    with TileContext(nc) as tc:
        with (
            tc.tile_pool(name="const", bufs=1) as cpool,
            tc.tile_pool(name="big", bufs=1) as big,
            tc.tile_pool(name="stream", bufs=4) as sp,
            tc.tile_pool(name="pagg", bufs=2, space="PSUM") as pa,
            tc.tile_pool(name="pdense", bufs=2, space="PSUM") as pdp,
            tc.tile_pool(name="pt2", bufs=3, space="PSUM") as pt,
        ):
            # Critical-path inputs (xs, bt1, w1) split across BOTH HW DMA
            # queues, interleaved in consumption order; bt2/w2/b2 deferred.
            bt1_sb = cpool.tile([128, NT, TS], F16)
            xs_sb = cpool.tile([128, NT, CIN], F16)
            w1_sb = cpool.tile([128, 2, CHID], F16)
            b1_sb = cpool.tile([128, 4], F32)
            bt2_sb = cpool.tile([128, NT, TS], F16)
            w2_sb = cpool.tile([128, 4, COUT], F16)
            b2_sb = cpool.tile([128, 2], F32)

            nc.sync.dma_start(xs_sb[:, 0:3, :], xs_d[:, 0:3, :])
            nc.scalar.dma_start(bt1_sb[:, 0:4, :], bt1_d[:, 0:4, :])
            nc.scalar.dma_start(xs_sb[:, 3:6, :], xs_d[:, 3:6, :])
            nc.sync.dma_start(w1_sb, w1_d[:, :, :])
            nc.sync.dma_start(b1_sb, b1_d[:, :])
            nc.scalar.dma_start(bt1_sb[:, 4:11, :], bt1_d[:, 4:11, :])
            nc.sync.dma_start(xs_sb[:, 6:9, :], xs_d[:, 6:9, :])
            nc.scalar.dma_start(xs_sb[:, 9:12, :], xs_d[:, 9:12, :])
            nc.sync.dma_start(xs_sb[:, 12:15, :], xs_d[:, 12:15, :])
            nc.scalar.dma_start(bt1_sb[:, 11:NT, :], bt1_d[:, 11:NT, :])
            nc.sync.dma_start(xs_sb[:, 15:NT, :], xs_d[:, 15:NT, :])
            nc.sync.dma_start(w2_sb, w2_d[:, :, :])
            nc.scalar.dma_start(bt2_sb, bt2_d[:, :, :])
            nc.scalar.dma_start(b2_sb, b2_d[:, :])

            a1T = big.tile([128, 2, NCOLS], F16)
            hT = big.tile([128, 4, NH], F16)
            outT_sb = big.tile([128, 2, HALF], F16)
            warm = cpool.tile([128, 512], F16)
            nc.gpsimd.memset(warm, 0.0)
            nc.gpsimd.memset(hT[:, :, 2056:NH], 0.0)

            # PE warm-up / filler matmuls: dependency-free work that keeps the
            # PE busy through input-DMA waits so the p-state clock stays
            # ramped. One dedicated psum tile (WAW-serialized on the PE only).
            wps = pa.tile([128, 480], F32, tag="warm", space="PSUM", bufs=1)

            def filler(n=1, w=480):
                for _ in range(n):
                    nc.tensor.matmul(wps[:, 0:w], lhsT=warm[:, 0:128],
                                     rhs=warm[:, 0:w], start=True, stop=True)

            filler(5)

            # ---------------- L1 aggregation (chan-major out) interleaved
            # with dense1. Pair pr drains a1T cols [480*pr/2 ...); a 480-col
            # dense1 block aligns exactly with two drained pairs, so the PE
            # always has dependency-ready work while xs chunks stream in.
            def agg1_pair(pr):
                # pair 8's second tile only feeds 16 live columns (ranks past
                # r0+2052 are never consumed by layer 2)
                wv = [TS, TS if pr < 8 else 16]
                ps = pa.tile([128, 2, 2 * TS], F32, tag="agg", space="PSUM",
                             name=f"agg1_{pr}")
                for dt_ in range(2):
                    t = 2 * pr + dt_
                    for cc in range(2):
                        nc.tensor.matmul(
                            ps[:, cc, TS * dt_:TS * dt_ + wv[dt_]],
                            lhsT=xs_sb[:, t, 128 * cc:128 * (cc + 1)],
                            rhs=bt1_sb[:, t, 0:wv[dt_]], start=True, stop=True)
                wtot = TS + wv[1]
                nc.vector.tensor_copy(
                    a1T[:, :, 2 * TS * pr:2 * TS * pr + wtot],
                    ps[:, :, 0:wtot])

            def dense1_block(lo, hi):
                for mb in range(4):
                    ps = pdp.tile([128, 480], F32, tag="d1", space="PSUM",
                                  name=f"d1_{lo}_{mb}")
                    for kb in range(2):
                        nc.tensor.matmul(
                            ps[:, 0:hi - lo],
                            lhsT=w1_sb[:, kb, 128 * mb:128 * (mb + 1)],
                            rhs=a1T[:, kb, lo:hi],
                            start=(kb == 0), stop=(kb == 1))
                    if mb % 2 == 0:
                        nc.scalar.activation(hT[:, mb, lo:hi], ps[:, 0:hi - lo],
                                             Relu, bias=b1_sb[:, mb:mb + 1],
                                             scale=1.0)
                    else:
                        nc.vector.tensor_scalar(
                            out=hT[:, mb, lo:hi], in0=ps[:, 0:hi - lo],
                            scalar1=b1_sb[:, mb:mb + 1], scalar2=0.0,
                            op0=add, op1=amax)

            # depth-1 software pipeline: the PE reaches dense1 block k-1 only
            # after pair 2k/2k+1, so the block's a1T drains are long done.
            # Fillers bridge xs-chunk arrival gaps without resetting the ramp.
            # small ramp-keeper fillers at every stall-prone pair boundary
            for k in range(5):
                agg1_pair(2 * k)
                filler(2, w=128)
                if 2 * k + 1 < 9:
                    agg1_pair(2 * k + 1)
                if k >= 1:
                    dense1_block(480 * (k - 1), 480 * k)
                    filler(2, w=128)
            dense1_block(480 * 4, 2056)

            # ---------------- dense2 (node-major window tiles) + L2 agg
            # (chan-major out) + bias/relu drains + streamed output DMA
            outT_ap = outT_d.rearrange("(c p) n -> p c n", p=128)
            ochunks = [(0, 384), (384, 768), (768, 1152), (1152, 1536),
                       (1536, 1920), (1920, HALF)]
            t2us = [None] * NT
            ph = [None]

            def dense2_tile(u):
                pst = pt.tile([128, COUT], F32, tag="t2", space="PSUM",
                              name=f"pst_{u}")
                for kb in range(4):
                    nc.tensor.matmul(
                        pst,
                        lhsT=hT[:, kb, TS * u:TS * u + 128],
                        rhs=w2_sb[:, kb, :],
                        start=(kb == 0), stop=(kb == 3))
                t2u = sp.tile([128, COUT], F16, tag="t2s", name=f"t2u_{u}")
                if u % 2 == 0:
                    nc.vector.tensor_copy(t2u, pst)
                else:
                    nc.scalar.activation(t2u, pst, Copy)
                t2us[u] = t2u

            def agg2_tile(u):
                if u % 2 == 0:
                    ph[0] = pa.tile([128, 2, 2 * TS], F32, tag="agg",
                                    space="PSUM", name=f"po_{u}")
                po = ph[0]
                for cc in range(2):
                    nc.tensor.matmul(
                        po[:, cc, TS * (u % 2):TS * (u % 2 + 1)],
                        lhsT=t2us[u][:, 128 * cc:128 * (cc + 1)],
                        rhs=bt2_sb[:, u, :], start=True, stop=True)
                if u % 2 == 1:
                    pr = u // 2
                    w_ = 2 * TS if u < NT - 1 else HALF - 2 * TS * pr
                    nc.scalar.activation(
                        outT_sb[:, 0, 2 * TS * pr:2 * TS * pr + w_],
                        po[:, 0, 0:w_], Relu,
                        bias=b2_sb[:, 0:1], scale=1.0)
                    nc.vector.tensor_scalar(
                        out=outT_sb[:, 1, 2 * TS * pr:2 * TS * pr + w_],
                        in0=po[:, 1, 0:w_],
                        scalar1=b2_sb[:, 1:2], scalar2=0.0,
                        op0=add, op1=amax)
                    done = 240 * (pr + 1) if u < NT - 1 else HALF
                    for lo_o, hi_o in ochunks:
                        if done >= hi_o and 240 * pr < hi_o:
                            # tail chunks issue from scalar (same engine as
                            # the last drain) to skip a cross-engine sem hop
                            eng = nc.scalar if lo_o >= 1536 else nc.sync
                            eng.dma_start(
                                outT_ap[:, :, lo_o:hi_o],
                                outT_sb[:, :, lo_o:hi_o])

            # depth-2 pipeline: two dense2 tiles stay in flight ahead of the
            # agg2 consumer, hiding the t2u psum-drain + semaphore latency.
            for u in range(NT):
                dense2_tile(u)
                if u >= 2:
                    agg2_tile(u - 2)
            agg2_tile(NT - 2)
            agg2_tile(NT - 1)

    nc.compile()
    return nc


# ---------------------------------------------------------------- host glue
def make_in_maps(density_maps, feature_maps, W1, b1, W2, b2):
    per_core, orders = _host_graph(density_maps)
    fm = np.asarray(feature_maps, dtype=np.float32).reshape(B, CIN, N)
    w1b = np.asarray(W1, np.float32).reshape(2, 128, CHID) \
        .transpose(1, 0, 2).astype(np.float16)
    w2b = np.asarray(W2, np.float32).reshape(4, 128, COUT) \
        .transpose(1, 0, 2).astype(np.float16)
    b1v = np.ascontiguousarray(np.asarray(b1, np.float32).reshape(4, 128).T)
    b2v = np.ascontiguousarray(np.asarray(b2, np.float32).reshape(2, 128).T)

    in_maps = []
    for c in range(8):
        g = per_core[c]
        fmT = fm[c // 2].T                      # [N, CIN]
        xs = fmT[g["node"]].astype(np.float16)  # [128, NT, CIN]
        in_maps.append({
            "xs": np.ascontiguousarray(xs),
            "bt1": np.ascontiguousarray(g["bt1"]),
            "bt2": np.ascontiguousarray(g["bt2"]),
            "w1b": w1b, "w2b": w2b, "b1v": b1v, "b2v": b2v,
        })
    return in_maps, orders


def kernel(density_maps, feature_maps, W1, b1, W2, b2):
    from concourse.bass_utils import run_bass_kernel_spmd

    if "nc" not in _COMPILED:
        _COMPILED["nc"] = build_nc()
    nc = _COMPILED["nc"]

    in_maps, orders = make_in_maps(density_maps, feature_maps, W1, b1, W2, b2)
    res = run_bass_kernel_spmd(nc, in_maps, core_ids=list(range(8)))

    out = np.empty((B, COUT, N), dtype=np.float32)
    for c in range(8):
        b, half = divmod(c, 2)
        r0 = half * HALF
        out[b][:, orders[b][r0:r0 + HALF]] = \
            res.results[c]["outT"].astype(np.float32)
    return np.ascontiguousarray(out.reshape(B, COUT, H, W))


# revision 39
# speedup vs baseline: 1.0134x; 1.0134x over previous
"""Trainium2 Bass kernel for DensityGCNProcessor.

Model: 2-layer GCN over a per-sample kNN graph built from 1-D density values
(K=4 nearest by |density_i - density_j|), symmetric deg^-1/2 normalization on
target indegree, relu after each layer.

Strategy
--------
kNN in a 1-D metric means: after sorting nodes by density, every node's 4
nearest neighbours lie within +/-4 sorted positions, so aggregation is a
9-diagonal banded matrix in sorted order. The host does all index math
(argsort, band weights with exact reference tie-breaking) and also lays the
features out in sorted order, pre-tiled for the device: overlapping window
tiles of 128 sorted nodes at stride 120, so each band aggregation is a single
k=128 matmul (no halo matmul).

Device pipeline per core (all matmuls fp16, psum fp32):
  1. agg1  (chan-major): A1^T[cin,:] tiles = xs_tile^T @ bandT1_tile
  2. dense1: H^T = relu(W1^T A1^T + b1)   (chid-major, scalar/vector drains)
  3. dense2: T2 window tiles = (hT cols)^T @ W2   (node-major)
  4. agg2  (chan-major): out^T = relu(T2_tile^T @ bandT2_tile + b2)
  5. linear DMA of out^T [256, 2048]; host scatters columns back to the
     original node order while unsharding.

Sharding: 8 cores = 4 batches x 2 rank-halves. Core c handles batch c//2,
sorted ranks [ (c%2)*2048, (c%2)*2048+2048 ).
"""

import numpy as np

# ---------------------------------------------------------------- constants
B = 4
CIN = 256
CHID = 512
COUT = 256
H = W = 64
N = H * W            # 4096 nodes per batch
KNN = 4
BAND = 4             # kNN lies within +/-4 sorted positions
HALF = N // 2        # 2048 ranks per core
NT = 18              # window tiles (128 rows, stride 120)
TS = 120             # out columns per tile
NCOLS = NT * TS      # 2160 hT columns computed
NH = 2176            # hT allocated columns (tail zeroed)

_COMPILED = {}


# ---------------------------------------------------------------- host graph
def _build_band_weights(d_flat):
    """order [N], w9 [N, 9] f32: out_s[r] = sum_o w9[r, o+4] * g_s[r+o]."""
    order = np.argsort(d_flat, kind="stable")
    d_s = d_flat[order]

    offs = np.arange(-BAND, BAND + 1)
    ridx = np.arange(N)[:, None] + offs[None, :]
    valid = (ridx >= 0) & (ridx < N)
    ridx_c = np.clip(ridx, 0, N - 1)
    c = np.abs(d_s[ridx_c] - d_s[:, None]).astype(np.float32)
    c = np.where(valid, c, np.float32(np.inf))
    cand_j = np.where(valid, order[ridx_c], N)

    # reference = stable argsort over the full row: ties by smaller orig index.
    sel = np.lexsort((cand_j, c), axis=1)
    tgt_s = np.take_along_axis(ridx_c, sel[:, 1:KNN + 1], axis=1).reshape(-1)
    src_s = np.repeat(np.arange(N), KNN)

    deg = np.ones(N, dtype=np.float32)
    np.add.at(deg, tgt_s, np.float32(1.0))
    dinv = (np.float32(1.0) / np.sqrt(deg)).astype(np.float32)

    m = np.zeros((N, 9), dtype=np.float32)
    np.add.at(m, (tgt_s, src_s - tgt_s + BAND), np.float32(1.0))
    m[:, BAND] += 1.0  # self loops

    ro = np.arange(N)[:, None] + offs[None, :]
    rov = (ro >= 0) & (ro < N)
    w9 = m * dinv[:, None] * dinv[np.clip(ro, 0, N - 1)] * rov
    return order.astype(np.int64), w9.astype(np.float32)


def _host_graph(density_maps):
    """Per-core tensors. Returns list of 8 dicts + per-batch orders."""
    pidx = np.arange(128)[:, None, None]          # window row
    tidx = np.arange(NT)[None, :, None]           # tile
    ridx = np.arange(TS)[None, None, :]           # out col within tile
    oo = pidx - ridx                              # w9 column (offset + 4)
    ok_o = (oo >= 0) & (oo <= 8)
    oo_c = np.clip(oo, 0, 8)

    per_core, orders = [], []
    for b in range(B):
        d = np.asarray(density_maps[b]).reshape(N).astype(np.float32)
        order, w9 = _build_band_weights(d)
        orders.append(order)
        for half in range(2):
            r0 = half * HALF

            # layer-1 band tiles: out rank = r0 - 4 + 120 t + r
            rank1 = r0 - 4 + TS * tidx + ridx
            ok1 = ok_o & (rank1 >= 0) & (rank1 < N)
            bt1 = np.where(ok1, w9[np.clip(rank1, 0, N - 1), oo_c], 0.0)

            # layer-2 band tiles: out rank = r0 + 120 t + r, only first 2048
            rank2 = r0 + TS * tidx + ridx
            ok2 = ok_o & (TS * tidx + ridx < HALF) & (rank2 < N)
            bt2 = np.where(ok2, w9[np.clip(rank2, 0, N - 1), oo_c], 0.0)

            # sorted feature window tiles: row p of tile t = rank r0-8+120t+p
            gi = r0 - 8 + TS * np.arange(NT)[None, :] + np.arange(128)[:, None]
            node = order[np.clip(gi, 0, N - 1)]   # [128, NT]

            per_core.append(dict(
                bt1=bt1.astype(np.float16),
                bt2=bt2.astype(np.float16),
                node=node,
            ))
    return per_core, orders


# ---------------------------------------------------------------- device IR
def build_nc():
    import concourse.bacc as bacc
    import concourse.mybir as mybir
    from concourse.tile import TileContext

    F32 = mybir.dt.float32
    F16 = mybir.dt.float16
    Relu = mybir.ActivationFunctionType.Relu
    Copy = mybir.ActivationFunctionType.Copy
    add = mybir.AluOpType.add
    amax = mybir.AluOpType.max

    nc = bacc.Bacc()
    xs_d = nc.dram_tensor("xs", [128, NT, CIN], F16, kind="ExternalInput")
    bt1_d = nc.dram_tensor("bt1", [128, NT, TS], F16, kind="ExternalInput")
    bt2_d = nc.dram_tensor("bt2", [128, NT, TS], F16, kind="ExternalInput")
    w1_d = nc.dram_tensor("w1b", [128, 2, CHID], F16, kind="ExternalInput")
    w2_d = nc.dram_tensor("w2b", [128, 4, COUT], F16, kind="ExternalInput")
    b1_d = nc.dram_tensor("b1v", [128, 4], F32, kind="ExternalInput")
    b2_d = nc.dram_tensor("b2v", [128, 2], F32, kind="ExternalInput")
    outT_d = nc.dram_tensor("outT", [COUT, HALF], F16, kind="ExternalOutput")

    with TileContext(nc) as tc:
        with (
            tc.tile_pool(name="const", bufs=1) as cpool,
            tc.tile_pool(name="big", bufs=1) as big,
            tc.tile_pool(name="stream", bufs=4) as sp,
            tc.tile_pool(name="pagg", bufs=2, space="PSUM") as pa,
            tc.tile_pool(name="pdense", bufs=2, space="PSUM") as pdp,
            tc.tile_pool(name="pt2", bufs=3, space="PSUM") as pt,
        ):
            # Critical-path inputs (xs, bt1, w1) split across BOTH HW DMA
            # queues, interleaved in consumption order; bt2/w2/b2 deferred.
            bt1_sb = cpool.tile([128, NT, TS], F16)
            xs_sb = cpool.tile([128, NT, CIN], F16)
            w1_sb = cpool.tile([128, 2, CHID], F16)
            b1_sb = cpool.tile([128, 4], F32)
            bt2_sb = cpool.tile([128, NT, TS], F16)
            w2_sb = cpool.tile([128, 4, COUT], F16)
            b2_sb = cpool.tile([128, 2], F32)

            nc.sync.dma_start(xs_sb[:, 0:3, :], xs_d[:, 0:3, :])
            nc.scalar.dma_start(bt1_sb[:, 0:4, :], bt1_d[:, 0:4, :])
            nc.scalar.dma_start(xs_sb[:, 3:6, :], xs_d[:, 3:6, :])
            nc.sync.dma_start(w1_sb, w1_d[:, :, :])
            nc.sync.dma_start(b1_sb, b1_d[:, :])
            nc.scalar.dma_start(bt1_sb[:, 4:11, :], bt1_d[:, 4:11, :])
            nc.sync.dma_start(xs_sb[:, 6:9, :], xs_d[:, 6:9, :])
            nc.scalar.dma_start(xs_sb[:, 9:12, :], xs_d[:, 9:12, :])
            nc.sync.dma_start(xs_sb[:, 12:15, :], xs_d[:, 12:15, :])
            nc.scalar.dma_start(bt1_sb[:, 11:NT, :], bt1_d[:, 11:NT, :])
            nc.sync.dma_start(xs_sb[:, 15:NT, :], xs_d[:, 15:NT, :])
            nc.sync.dma_start(w2_sb, w2_d[:, :, :])
            nc.scalar.dma_start(bt2_sb, bt2_d[:, :, :])
            nc.scalar.dma_start(b2_sb, b2_d[:, :])

            a1T = big.tile([128, 2, NCOLS], F16)
            hT = big.tile([128, 4, NH], F16)
            outT_sb = big.tile([128, 2, HALF], F16)
            warm = cpool.tile([128, 512], F16)
            nc.gpsimd.memset(warm, 0.0)
            nc.gpsimd.memset(hT[:, :, 2056:NH], 0.0)

            # PE warm-up / filler matmuls: dependency-free work that keeps the
            # PE busy through input-DMA waits so the p-state clock stays
            # ramped. One dedicated psum tile (WAW-serialized on the PE only).
            wps = pa.tile([128, 480], F32, tag="warm", space="PSUM", bufs=1)

            def filler(n=1, w=480):
                for _ in range(n):
                    nc.tensor.matmul(wps[:, 0:w], lhsT=warm[:, 0:128],
                                     rhs=warm[:, 0:w], start=True, stop=True)

            filler(5)

            # ---------------- L1 aggregation (chan-major out) interleaved
            # with dense1. Pair pr drains a1T cols [480*pr/2 ...); a 480-col
            # dense1 block aligns exactly with two drained pairs, so the PE
            # always has dependency-ready work while xs chunks stream in.
            def agg1_pair(pr):
                # pair 8's second tile only feeds 16 live columns (ranks past
                # r0+2052 are never consumed by layer 2)
                wv = [TS, TS if pr < 8 else 16]
                ps = pa.tile([128, 2, 2 * TS], F32, tag="agg", space="PSUM",
                             name=f"agg1_{pr}")
                for dt_ in range(2):
                    t = 2 * pr + dt_
                    for cc in range(2):
                        nc.tensor.matmul(
                            ps[:, cc, TS * dt_:TS * dt_ + wv[dt_]],
                            lhsT=xs_sb[:, t, 128 * cc:128 * (cc + 1)],
                            rhs=bt1_sb[:, t, 0:wv[dt_]], start=True, stop=True)
                wtot = TS + wv[1]
                nc.vector.tensor_copy(
                    a1T[:, :, 2 * TS * pr:2 * TS * pr + wtot],
                    ps[:, :, 0:wtot])

            def dense1_block(lo, hi):
                for mb in range(4):
                    ps = pdp.tile([128, 480], F32, tag="d1", space="PSUM",
                                  name=f"d1_{lo}_{mb}")
                    for kb in range(2):
                        nc.tensor.matmul(
                            ps[:, 0:hi - lo],
                            lhsT=w1_sb[:, kb, 128 * mb:128 * (mb + 1)],
                            rhs=a1T[:, kb, lo:hi],
                            start=(kb == 0), stop=(kb == 1))
                    if mb % 2 == 0:
                        nc.scalar.activation(hT[:, mb, lo:hi], ps[:, 0:hi - lo],
                                             Relu, bias=b1_sb[:, mb:mb + 1],
                                             scale=1.0)
                    else:
                        nc.vector.tensor_scalar(
                            out=hT[:, mb, lo:hi], in0=ps[:, 0:hi - lo],
                            scalar1=b1_sb[:, mb:mb + 1], scalar2=0.0,
                            op0=add, op1=amax)

            # depth-1 software pipeline: the PE reaches dense1 block k-1 only
            # after pair 2k/2k+1, so the block's a1T drains are long done.
            # Fillers bridge xs-chunk arrival gaps without resetting the ramp.
            # small ramp-keeper fillers in the xs-chunk wait window only
            for k in range(5):
                agg1_pair(2 * k)
                if k >= 1:
                    filler(2, w=128)
                if 2 * k + 1 < 9:
                    agg1_pair(2 * k + 1)
                if k >= 1:
                    dense1_block(480 * (k - 1), 480 * k)
            dense1_block(480 * 4, 2056)

            # ---------------- dense2 (node-major window tiles) + L2 agg
            # (chan-major out) + bias/relu drains + streamed output DMA
            outT_ap = outT_d.rearrange("(c p) n -> p c n", p=128)
            ochunks = [(0, 384), (384, 768), (768, 1152), (1152, 1536),
                       (1536, 1920), (1920, HALF)]
            t2us = [None] * NT
            ph = [None]

            def dense2_tile(u):
                pst = pt.tile([128, COUT], F32, tag="t2", space="PSUM",
                              name=f"pst_{u}")
                for kb in range(4):
                    nc.tensor.matmul(
                        pst,
                        lhsT=hT[:, kb, TS * u:TS * u + 128],
                        rhs=w2_sb[:, kb, :],
                        start=(kb == 0), stop=(kb == 3))
                t2u = sp.tile([128, COUT], F16, tag="t2s", name=f"t2u_{u}")
                if u % 2 == 0:
                    nc.vector.tensor_copy(t2u, pst)
                else:
                    nc.scalar.activation(t2u, pst, Copy)
                t2us[u] = t2u

            def agg2_tile(u):
                if u % 2 == 0:
                    ph[0] = pa.tile([128, 2, 2 * TS], F32, tag="agg",
                                    space="PSUM", name=f"po_{u}")
                po = ph[0]
                for cc in range(2):
                    nc.tensor.matmul(
                        po[:, cc, TS * (u % 2):TS * (u % 2 + 1)],
                        lhsT=t2us[u][:, 128 * cc:128 * (cc + 1)],
                        rhs=bt2_sb[:, u, :], start=True, stop=True)
                if u % 2 == 1:
                    pr = u // 2
                    w_ = 2 * TS if u < NT - 1 else HALF - 2 * TS * pr
                    nc.scalar.activation(
                        outT_sb[:, 0, 2 * TS * pr:2 * TS * pr + w_],
                        po[:, 0, 0:w_], Relu,
                        bias=b2_sb[:, 0:1], scale=1.0)
                    nc.vector.tensor_scalar(
                        out=outT_sb[:, 1, 2 * TS * pr:2 * TS * pr + w_],
                        in0=po[:, 1, 0:w_],
                        scalar1=b2_sb[:, 1:2], scalar2=0.0,
                        op0=add, op1=amax)
                    done = 240 * (pr + 1) if u < NT - 1 else HALF
                    for lo_o, hi_o in ochunks:
                        if done >= hi_o and 240 * pr < hi_o:
                            nc.sync.dma_start(
                                outT_ap[:, :, lo_o:hi_o],
                                outT_sb[:, :, lo_o:hi_o])

            # depth-2 pipeline: two dense2 tiles stay in flight ahead of the
            # agg2 consumer, hiding the t2u psum-drain + semaphore latency.
            for u in range(NT):
                dense2_tile(u)
                if u >= 2:
                    agg2_tile(u - 2)
            agg2_tile(NT - 2)
            agg2_tile(NT - 1)

    nc.compile()
    return nc


# ---------------------------------------------------------------- host glue
def make_in_maps(density_maps, feature_maps, W1, b1, W2, b2):
    per_core, orders = _host_graph(density_maps)
    fm = np.asarray(feature_maps, dtype=np.float32).reshape(B, CIN, N)
    w1b = np.asarray(W1, np.float32).reshape(2, 128, CHID) \
        .transpose(1, 0, 2).astype(np.float16)
    w2b = np.asarray(W2, np.float32).reshape(4, 128, COUT) \
        .transpose(1, 0, 2).astype(np.float16)
    b1v = np.ascontiguousarray(np.asarray(b1, np.float32).reshape(4, 128).T)
    b2v = np.ascontiguousarray(np.asarray(b2, np.float32).reshape(2, 128).T)

    in_maps = []
    for c in range(8):
        g = per_core[c]
        fmT = fm[c // 2].T                      # [N, CIN]
        xs = fmT[g["node"]].astype(np.float16)  # [128, NT, CIN]
        in_maps.append({
            "xs": np.ascontiguousarray(xs),
            "bt1": np.ascontiguousarray(g["bt1"]),
            "bt2": np.ascontiguousarray(g["bt2"]),
            "w1b": w1b, "w2b": w2b, "b1v": b1v, "b2v": b2v,
        })
    return in_maps, orders


def kernel(density_maps, feature_maps, W1, b1, W2, b2):
    from concourse.bass_utils import run_bass_kernel_spmd

    if "nc" not in _COMPILED:
        _COMPILED["nc"] = build_nc()
    nc = _COMPILED["nc"]

    in_maps, orders = make_in_maps(density_maps, feature_maps, W1, b1, W2, b2)
    res = run_bass_kernel_spmd(nc, in_maps, core_ids=list(range(8)))

    out = np.empty((B, COUT, N), dtype=np.float32)
    for c in range(8):
        b, half = divmod(c, 2)
        r0 = half * HALF
        out[b][:, orders[b][r0:r0 + HALF]] = \
            res.results[c]["outT"].astype(np.float32)
    return np.ascontiguousarray(out.reshape(B, COUT, H, W))
